# revision 5
# baseline (speedup 1.0000x reference)
"""3-layer GCN (PyG GCNConv-style) on 8 Trainium2 NeuronCores.

Strategy (graph/data parallel, per the sharding hint):
  - Nodes partitioned contiguously: 6272 per core (49 windows x 128 nodes;
    node n is owned by core n // 6272). Edges assigned to the core owning
    their destination; self-loops added host-side as ordinary edges.
  - Per layer: each core computes xw = h_own @ W for its own nodes (49
    matmuls off a feature-major hT kept in SBUF), AllGathers the full
    [50176, 64] xw table, then gathers xw[src] rows for its edges with
    dma_gather and performs the normalized scatter-add as matmul-
    accumulates into PSUM: for each 128-edge tile, a [128e x 128dst]
    "value matrix" (norm[e] at column dstcol[e], built on the Vector
    engine with a fused iota/is_equal/mult op) multiplies the gathered
    messages, accumulating a 128-node window's segment sum.
  - dma_gather uses int16 indices, so the 50176-row xw table is split in
    half (rows < 25088 = cores 0-3); each window's edge tiles are grouped
    lo-half first, then hi-half, and gathers run per half over groups of
    consecutive windows.
  - Window epilogue: +bias, sigmoid, agent-row tap (stride-4 partitions),
    and a PE transpose back into the next layer's feature-major hT.

Host-side work is limited to graph index preprocessing (degree counts,
edge normalization constants, sorting/padding edges by destination
window) and final output assembly.
"""

import sys

sys.path.insert(0, "/opt/trn_rl_repo")

import numpy as np

N_NODES = 50000
D = 64
N_CORES = 8
WSZ = 128               # dst-window size (PSUM partition dim)
NW = 49                 # windows per core
NPC = NW * WSZ          # 6272 padded nodes per core (50176 total >= 50000)
LO_ROWS = 4 * NPC       # 25088: table-half split (int16 index limit)
GRP = 2                 # windows per gather group


def _preprocess(edge_index):
    """Degree/norm computation and edge layout.

    Returns per-core device arrays plus the uniform tile schedule (shared
    by all cores: the program is SPMD, so tile counts per (window, half)
    are maxed over cores and padded with norm=0 edges).
    """
    src = np.asarray(edge_index[0], dtype=np.int64)
    dst = np.asarray(edge_index[1], dtype=np.int64)

    deg = np.bincount(dst, minlength=N_NODES).astype(np.float32) + 1.0
    dinv = (1.0 / np.sqrt(deg)).astype(np.float32)

    loop = np.arange(N_NODES, dtype=np.int64)
    s_all = np.concatenate([src, loop])
    d_all = np.concatenate([dst, loop])
    nrm = np.concatenate([dinv[src] * dinv[dst], dinv * dinv]).astype(np.float32)

    core = d_all // NPC
    local = d_all - core * NPC
    win = local // WSZ
    col = (local % WSZ).astype(np.float32)
    half = (s_all >= LO_ROWS).astype(np.int64)

    # group edges by (core, window, half)
    key = (core * NW + win) * 2 + half
    order = np.argsort(key, kind="stable")
    key_sorted = key[order]
    bounds = np.searchsorted(key_sorted, np.arange(N_CORES * NW * 2 + 1))
    cnt = (bounds[1:] - bounds[:-1]).reshape(N_CORES, NW, 2)

    # uniform tiles per (window, half), maxed over cores
    n_th = -(-cnt.max(axis=0) // 128)           # [NW, 2]
    n_th[:, 0] = np.maximum(n_th[:, 0], 1)      # >=1 tile so PSUM gets start=True
    T = int(n_th.sum())

    # tile stream: groups of GRP windows; within a group all lo tiles
    # (w ascending), then all hi tiles
    tile_win = []       # window of tile t
    tile_half = []
    runs = []           # (start_tile, n_tiles, half)
    win_tile_base = np.zeros((NW, 2), np.int64)
    for g0 in range(0, NW, GRP):
        ws = range(g0, min(g0 + GRP, NW))
        for h in (0, 1):
            r0 = len(tile_win)
            for w in ws:
                win_tile_base[w, h] = len(tile_win)
                tile_win += [w] * int(n_th[w, h])
                tile_half += [h] * int(n_th[w, h])
            if len(tile_win) > r0:
                runs.append((r0, len(tile_win) - r0, h))
    tile_win = np.asarray(tile_win)
    tile_half = np.asarray(tile_half)
    assert len(tile_win) == T

    # first/last tile of each window (for PSUM start/stop)
    win_first = np.full(NW, -1, np.int64)
    win_last = np.full(NW, -1, np.int64)
    for t in range(T):
        w = tile_win[t]
        if win_first[w] < 0:
            win_first[w] = t
        win_last[w] = t

    # fill per-core edge arrays
    idx_flat = np.zeros((N_CORES, T * 128), np.int16)
    col_arr = np.zeros((N_CORES, 128, T), np.float32)
    nrm_arr = np.zeros((N_CORES, 128, T), np.float32)
    for c in range(N_CORES):
        for w in range(NW):
            for h in (0, 1):
                gidx = (c * NW + w) * 2 + h
                e0, e1 = bounds[gidx], bounds[gidx + 1]
                n = e1 - e0
                if n == 0:
                    continue
                sel = order[e0:e1]
                base = win_tile_base[w, h] * 128
                pos = base + np.arange(n)
                idx_flat[c, pos] = (s_all[sel] - h * LO_ROWS).astype(np.int16)
                tt = pos // 128
                pp = pos % 128
                col_arr[c, pp, tt] = col[sel]
                nrm_arr[c, pp, tt] = nrm[sel]

    # wrap indices for dma_gather: [128, T*8] int16,
    # arr[p, t*8+c] = idx[t*128 + c*16 + (p % 16)]
    w16 = idx_flat.reshape(N_CORES, T, 8, 16).transpose(0, 3, 1, 2).reshape(
        N_CORES, 16, T * 8)
    idx_arr = np.tile(w16, (1, 8, 1))           # [N_CORES, 128, T*8]

    sched = dict(T=T, runs=runs, tile_win=tile_win, tile_half=tile_half,
                 win_first=win_first, win_last=win_last)
    return idx_arr, col_arr, nrm_arr, sched


def _build_program(sched):
    import concourse.bass as bass
    import concourse.bacc as bacc
    import concourse.tile as tile
    from concourse import mybir

    f32 = mybir.dt.float32
    i16 = mybir.dt.int16

    T = sched["T"]
    runs = sched["runs"]
    tile_win = sched["tile_win"]
    win_first = sched["win_first"]
    win_last = sched["win_last"]
    max_run = max(n for _, n, _ in runs)

    nc = bacc.Bacc("TRN2", target_bir_lowering=False, debug=False,
                   num_devices=N_CORES)

    xT_own = nc.dram_tensor("xT_own", [64, NPC], f32, kind="ExternalInput")
    src_idx = nc.dram_tensor("src_idx", [128, T * 8], i16, kind="ExternalInput")
    dstcol = nc.dram_tensor("dstcol", [128, T], f32, kind="ExternalInput")
    normv = nc.dram_tensor("normv", [128, T], f32, kind="ExternalInput")
    Wmat = nc.dram_tensor("Wmat", [3, 64, 64], f32, kind="ExternalInput")
    bias_bc = nc.dram_tensor("bias_bc", [3, 128, 64], f32, kind="ExternalInput")
    iota_in = nc.dram_tensor("iota", [128, 128], f32, kind="ExternalInput")
    ident_in = nc.dram_tensor("ident", [128, 128], f32, kind="ExternalInput")
    agents = nc.dram_tensor("agents_out", [3, NW * 32, 64], f32,
                            kind="ExternalOutput")

    with tile.TileContext(nc) as tc:
        with (
            tc.tile_pool(name="const", bufs=1) as constp,
            tc.tile_pool(name="hT", bufs=2) as hTp,
            tc.tile_pool(name="msg", bufs=3) as msgp,
            tc.tile_pool(name="vm", bufs=4) as vmp,
            tc.tile_pool(name="small", bufs=4) as smallp,
            tc.tile_pool(name="ps_seg", bufs=3, space="PSUM") as ps_seg,
            tc.tile_pool(name="ps_xw", bufs=2, space="PSUM") as ps_xw,
            tc.tile_pool(name="ps_tr", bufs=2, space="PSUM") as ps_tr,
            tc.tile_pool(name="dram_ag", bufs=2, space="DRAM") as dram_ag,
            tc.tile_pool(name="dram_xw", bufs=1, space="DRAM") as dram_xw,
        ):
            meta_idx = constp.tile([128, T * 8], i16)
            meta_col = constp.tile([128, T], f32)
            meta_nrm = constp.tile([128, T], f32)
            nc.sync.dma_start(out=meta_idx[:], in_=src_idx[:, :])
            nc.sync.dma_start(out=meta_col[:], in_=dstcol[:, :])
            nc.sync.dma_start(out=meta_nrm[:], in_=normv[:, :])
            iota_t = constp.tile([128, 128], f32)
            ident_t = constp.tile([128, 128], f32)
            nc.sync.dma_start(out=iota_t[:], in_=iota_in[:, :])
            nc.sync.dma_start(out=ident_t[:], in_=ident_in[:, :])
            w_tiles = []
            b_tiles = []
            for l in range(3):
                wt = constp.tile([64, 64], f32, name=f"w{l}")
                bt = constp.tile([128, 64], f32, name=f"b{l}")
                nc.sync.dma_start(out=wt[:], in_=Wmat[l, :, :])
                nc.sync.dma_start(out=bt[:], in_=bias_bc[l, :, :])
                w_tiles.append(wt)
                b_tiles.append(bt)

            hT_cur = hTp.tile([64, NPC], f32, tag="hT", name="hT0")
            nc.sync.dma_start(out=hT_cur[:], in_=xT_own[:, :])

            for l in range(3):
                # ---- own-shard linear: xw_own = h_own @ W_l ----
                ag_t = dram_ag.tile([NPC, 64], f32, tag="ag", name=f"ag{l}")
                for i in range(NW):
                    ps = ps_xw.tile([128, 64], f32, tag="psxw", name=f"psxw{l}_{i}")
                    nc.tensor.matmul(
                        out=ps[:],
                        lhsT=hT_cur[:, i * 128:(i + 1) * 128],
                        rhs=w_tiles[l][:],
                        start=True, stop=True,
                    )
                    st = smallp.tile([128, 64], f32, tag="xwst", name=f"st{l}_{i}")
                    nc.vector.tensor_copy(out=st[:], in_=ps[:])
                    nc.sync.dma_start(out=ag_t[i * 128:(i + 1) * 128, :], in_=st[:])

                xw_full = dram_xw.tile([N_CORES, NPC, 64], f32, tag="xwf",
                                       addr_space="Shared", name=f"xwf{l}")
                nc.gpsimd.collective_compute(
                    "AllGather",
                    mybir.AluOpType.bypass,
                    replica_groups=[list(range(N_CORES))],
                    ins=[ag_t.opt()],
                    outs=[xw_full.opt()],
                )
                xw_flat = xw_full[:].rearrange("a b c -> (a b) c")

                if l < 2:
                    hT_next = hTp.tile([64, NPC], f32, tag="hT", name=f"hT{l + 1}")
                else:
                    hT_next = None

                # ---- gather + windowed segment-sum ----
                win_ps = {}
                for r, (t0, nt, h) in enumerate(runs):
                    msg = msgp.tile([128, max_run * 64], f32, tag="msg",
                                    name=f"msg{l}_{r}")
                    src_half = (xw_flat[0:LO_ROWS, :] if h == 0
                                else xw_flat[LO_ROWS:2 * LO_ROWS, :])
                    nc.gpsimd.dma_gather(
                        out_ap=msg[:, :nt * 64].rearrange("p (k f) -> p k f", f=64),
                        in_ap=src_half,
                        idxs_ap=meta_idx[:, t0 * 8:(t0 + nt) * 8],
                        num_idxs=nt * 128,
                        num_idxs_reg=nt * 128,
                        elem_size=64,
                        single_packet=False,
                    )
                    for j in range(nt):
                        t = t0 + j
                        w = int(tile_win[t])
                        first = (t == win_first[w])
                        last = (t == win_last[w])
                        if first:
                            win_ps[w] = ps_seg.tile([128, 64], f32, tag="seg",
                                                    name=f"seg{l}_{w}")
                        cur_ps = win_ps[w]
                        vm = vmp.tile([128, 128], f32, tag="vm", name=f"vm{l}_{t}")
                        nc.vector.tensor_scalar(
                            out=vm[:],
                            in0=iota_t[:],
                            scalar1=meta_col[:, t:t + 1],
                            scalar2=meta_nrm[:, t:t + 1],
                            op0=mybir.AluOpType.is_equal,
                            op1=mybir.AluOpType.mult,
                        )
                        nc.tensor.matmul(
                            out=cur_ps[:],
                            lhsT=vm[:],
                            rhs=msg[:, j * 64:(j + 1) * 64],
                            start=first, stop=last,
                        )
                        if last:
                            hwin = smallp.tile([128, 64], f32, tag="hwin",
                                               name=f"hw{l}_{w}")
                            nc.vector.tensor_add(out=hwin[:], in0=cur_ps[:],
                                                 in1=b_tiles[l][:])
                            nc.scalar.activation(
                                out=hwin[:], in_=hwin[:],
                                func=mybir.ActivationFunctionType.Sigmoid,
                            )
                            nc.sync.dma_start(
                                out=agents[l, w * 32:(w + 1) * 32, :],
                                in_=hwin[0:128:4, :],
                            )
                            if hT_next is not None:
                                pt = ps_tr.tile([64, 128], f32, tag="tr",
                                                name=f"tr{l}_{w}")
                                nc.tensor.transpose(out=pt[:], in_=hwin[:],
                                                    identity=ident_t[:])
                                nc.vector.tensor_copy(
                                    out=hT_next[:, w * 128:(w + 1) * 128],
                                    in_=pt[:],
                                )
                hT_cur = hT_next

    nc.compile()
    return nc


def kernel(**inputs):
    from concourse import bass_utils

    x = np.asarray(inputs["x"], dtype=np.float32)
    edge_index = np.asarray(inputs["edge_index"])
    agent_idx = np.asarray(inputs["agent_idx"], dtype=np.int64)
    Ws = [np.asarray(inputs[f"W{i}"], dtype=np.float32) for i in range(3)]
    bs = [np.asarray(inputs[f"b{i}"], dtype=np.float32) for i in range(3)]

    idx_arr, col_arr, nrm_arr, sched = _preprocess(edge_index)

    nc = _build_program(sched)

    xpad = np.zeros((N_CORES * NPC, D), np.float32)
    xpad[:N_NODES] = x
    Wstack = np.ascontiguousarray(np.stack(Ws))
    bias_stack = np.ascontiguousarray(
        np.stack([np.tile(b[None, :], (128, 1)) for b in bs]))
    iota = np.tile(np.arange(128, dtype=np.float32)[None, :], (128, 1))
    ident = np.eye(128, dtype=np.float32)

    in_maps = []
    for c in range(N_CORES):
        in_maps.append({
            "xT_own": np.ascontiguousarray(xpad[c * NPC:(c + 1) * NPC].T),
            "src_idx": np.ascontiguousarray(idx_arr[c]),
            "dstcol": np.ascontiguousarray(col_arr[c]),
            "normv": np.ascontiguousarray(nrm_arr[c]),
            "Wmat": Wstack,
            "bias_bc": bias_stack,
            "iota": iota,
            "ident": ident,
        })

    res = bass_utils.run_bass_kernel_spmd(
        nc, in_maps, core_ids=list(range(N_CORES)))

    taps = np.stack([res.results[c]["agents_out"] for c in range(N_CORES)])
    # taps[c, l, r, :] = h_l for node (c*NPC + 4*r)
    n_agents = agent_idx.shape[0]
    out = np.empty((n_agents, 3 * D), np.float32)
    c_of = agent_idx // NPC
    r_of = (agent_idx % NPC) // 4
    for l in range(3):
        out[:, l * D:(l + 1) * D] = taps[c_of, l, r_of, :]
    return out
